# revision 2
# baseline (speedup 1.0000x reference)
"""Radix-2 Trainium2 kernel for CoherentDONN (v3).

A = (F2^H (x) I_256) . blockdiag(B2_0, B2_1) . (F2 (x) I_256)  (exact:
the circulant A commutes with shift-by-256).  All DFT2 combines have
REAL +/-1 coefficients - no re/im mixing anywhere in the glue.

Per layer: pre1 (4 paired wide adds) -> S1 (64 fp16 matmuls/img, N=256)
-> pre2 (8 psum adds/img) -> S2 -> post-n (8 psum adds/img) -> post-m
(4 paired adds) -> mask (6 paired muls w/ broadcast masks).
Glue ops process image PAIRS in one instruction where possible.
PE ~44 matmul-cycles-us/img dominates; DVE/Pool glue hides under it.
"""

import os
import numpy as np

import concourse.bass as bass
import concourse.mybir as mybir
import concourse.tile as tile
from concourse import bacc

N_CORES = int(os.environ.get("DONN_CORES", "8"))
PER_CORE = int(os.environ.get("DONN_IMG", str(128 // max(N_CORES, 1))))
RES = 512
NB = 128
NL = 3
NCLS = 10
LAMBDA = 5.32e-07
Z = 0.035
DX = 1e-06

f32 = mybir.dt.float32
f16 = mybir.dt.float16
MULT = mybir.AluOpType.mult
ADD = mybir.AluOpType.add
SUB = mybir.AluOpType.subtract
SQUARE = mybir.ActivationFunctionType.Square
COPY = mybir.ActivationFunctionType.Copy


def _make_A():
    fx = np.fft.fftfreq(RES, DX)
    h = np.exp(-1j * np.pi * LAMBDA * Z * fx**2)
    F = np.fft.fft(np.eye(RES))
    return F.conj().T @ np.diag(h) @ F / RES


def _host_constants():
    A = _make_A()
    F2 = np.array([[1, 1], [1, -1]], dtype=complex)
    T2 = np.kron(F2, np.eye(256))
    B2 = T2 @ A @ T2.conj().T / 4.0
    Bj = [B2[:256, :256], B2[256:, 256:]]
    # moving planes m2??[p, j, s, b] = Bj[j].T[s*128+p, b]
    m2re = np.zeros((NB, 2, 2, 256), np.float32)
    m2im = np.zeros((NB, 2, 2, 256), np.float32)
    for j in range(2):
        BT = Bj[j].T
        for s in range(2):
            m2re[:, j, s, :] = BT.real[s*NB:(s+1)*NB, :]
            m2im[:, j, s, :] = BT.imag[s*NB:(s+1)*NB, :]
    c16 = lambda m: np.ascontiguousarray(m, np.float16)
    return c16(m2re), c16(m2im), c16(-m2im)


def _build(nc_cache={}):
    if "nc" in nc_cache:
        return nc_cache["nc"], None

    nc = bacc.Bacc("TRN2", target_bir_lowering=False, debug=False,
                   num_devices=N_CORES)

    x_d = nc.dram_tensor("x", [PER_CORE, NB, 4, RES], f16, kind="ExternalInput").ap()
    m2re_d = nc.dram_tensor("m2re", [NB, 2, 2, 256], f16, kind="ExternalInput").ap()
    m2im_d = nc.dram_tensor("m2im", [NB, 2, 2, 256], f16, kind="ExternalInput").ap()
    m2imn_d = nc.dram_tensor("m2imn", [NB, 2, 2, 256], f16, kind="ExternalInput").ap()
    pc_d = nc.dram_tensor("pcos", [NL, NB, 4, RES], f16, kind="ExternalInput").ap()
    ps_d = nc.dram_tensor("psin", [NL, NB, 4, RES], f16, kind="ExternalInput").ap()
    fcw_d = nc.dram_tensor("fcw", [NB, NCLS, 4 * RES], f16, kind="ExternalInput").ap()
    fcb_d = nc.dram_tensor("fcb", [PER_CORE, NCLS], f32, kind="ExternalInput").ap()
    out_d = nc.dram_tensor("out", [PER_CORE, NCLS], f32, kind="ExternalOutput").ap()

    with tile.TileContext(nc) as tc:
        with tc.tile_pool(name="consts", bufs=1) as constp, \
             tc.tile_pool(name="dram", bufs=1, space="DRAM") as dramp:
            pcos, psin = [], []
            for l in range(NL):
                ct = constp.tile([NB, 4, RES], f16, tag=f"pc{l}")
                st = constp.tile([NB, 4, RES], f16, tag=f"ps{l}")
                pcos.append(ct)
                psin.append(st)
            # layer-0 masks first so entry can start immediately
            nc.sync.dma_start(pcos[0][:], pc_d[0])
            nc.sync.dma_start(psin[0][:], ps_d[0])
            m2re = constp.tile([NB, 2, 2, 256], f16, tag="m2re")
            m2im = constp.tile([NB, 2, 2, 256], f16, tag="m2im")
            m2imn = constp.tile([NB, 2, 2, 256], f16, tag="m2imn")
            fcb_t = constp.tile([PER_CORE, NCLS], f32, tag="fcb")

            def late_const_dmas():
                for t, d in ((m2re, m2re_d), (m2im, m2im_d), (m2imn, m2imn_d)):
                    nc.sync.dma_start(t[:], d[:])
                for l in range(1, NL):
                    nc.sync.dma_start(pcos[l][:], pc_d[l])
                    nc.sync.dma_start(psin[l][:], ps_d[l])
                nc.sync.dma_start(fcb_t[:], fcb_d[:])
            featd = dramp.tile([NB, PER_CORE, 4 * RES], f16)

            def bc(ap):
                """broadcast a [128, 4, 512] const AP across the img axis."""
                return ap.unsqueeze(1).broadcast_to([NB, 2, 4, RES])

            with tc.tile_pool(name="xp", bufs=1) as xpool, \
                 tc.tile_pool(name="vp", bufs=2) as vpool, \
                 tc.tile_pool(name="pp", bufs=1) as ppool, \
                 tc.tile_pool(name="qp", bufs=2) as qpool, \
                 tc.tile_pool(name="up", bufs=1) as upool, \
                 tc.tile_pool(name="uu", bufs=1) as uupool, \
                 tc.tile_pool(name="sc", bufs=1) as scp, \
                 tc.tile_pool(name="fcw", bufs=2) as fcwp, \
                 tc.tile_pool(name="fcr", bufs=2) as fcrp, \
                 tc.tile_pool(name="fco", bufs=1) as fcop, \
                 tc.tile_pool(name="ps", bufs=8, space="PSUM") as psum:

                def pre1(vre, vim):
                    """paired DFT2 over c: P_j = V_lo +/- V_hi.
                    P tiles [128, 2img, 2j, 2sub, 512]."""
                    Pre = ppool.tile([NB, 2, 2, 2, RES], f16, tag="p1r")
                    Pim = ppool.tile([NB, 2, 2, 2, RES], f16, tag="p1i")
                    nc.vector.tensor_tensor(Pre[:, :, 0, :, :], vre[:, :, 0:2, :], vre[:, :, 2:4, :], ADD)
                    nc.vector.tensor_tensor(Pim[:, :, 0, :, :], vim[:, :, 0:2, :], vim[:, :, 2:4, :], ADD)
                    nc.vector.tensor_tensor(Pre[:, :, 1, :, :], vre[:, :, 0:2, :], vre[:, :, 2:4, :], SUB)
                    nc.vector.tensor_tensor(Pim[:, :, 1, :, :], vim[:, :, 0:2, :], vim[:, :, 2:4, :], SUB)
                    return Pre, Pim

                def mm_stage(slicer):
                    """One radix-2 stage; slicer(j, s, a) -> (re, im) lhsT
                    slices. Bank order 0,2,1,3 x (re,im)."""
                    banks = {}
                    for a in (0, 2, 1, 3):
                        pr = psum.tile([NB, RES], f32, tag="bk")
                        pi = psum.tile([NB, RES], f32, tag="bk")
                        for j in range(2):
                            dst = pr[:, bass.ts(j, 256)]
                            dsti = pi[:, bass.ts(j, 256)]
                            for s in range(2):
                                ls, li = slicer(j, s, a)
                                nc.tensor.matmul(dst, ls, m2re[:, j, s, :],
                                                 start=(s == 0), stop=False)
                                nc.tensor.matmul(dsti, ls, m2im[:, j, s, :],
                                                 start=(s == 0), stop=False)
                            for s in range(2):
                                ls, li = slicer(j, s, a)
                                nc.tensor.matmul(dst, li, m2imn[:, j, s, :],
                                                 start=False, stop=(s == 1))
                                nc.tensor.matmul(dsti, li, m2re[:, j, s, :],
                                                 start=False, stop=(s == 1))
                        banks[a] = (pr, pi)
                    return banks

                def drain(banks, dre, dim):
                    """Act: psum banks -> per-image fp16 tiles [128,4a,512]."""
                    for a in (0, 2, 1, 3):
                        nc.scalar.activation(dre[:, a, :], banks[a][0][:], COPY)
                        nc.scalar.activation(dim[:, a, :], banks[a][1][:], COPY)

                def img_dft2(dre, dim, dst_re, dst_im):
                    """per-image +/- combines of drained bank pairs (0,2),(1,3):
                    dst[j2][s] slices [128, 512]."""
                    for s, (lo, hi) in enumerate(((0, 2), (1, 3))):
                        nc.gpsimd.tensor_tensor(dst_re[0][s], dre[:, lo, :], dre[:, hi, :], ADD)
                        nc.vector.tensor_tensor(dst_im[0][s], dim[:, lo, :], dim[:, hi, :], ADD)
                        nc.gpsimd.tensor_tensor(dst_re[1][s], dre[:, lo, :], dre[:, hi, :], SUB)
                        nc.vector.tensor_tensor(dst_im[1][s], dim[:, lo, :], dim[:, hi, :], SUB)

                def pre2_img(P2re, P2im, dre, dim):
                    # P2 per-image [128, 2j2, 2s, 512]
                    img_dft2(dre, dim,
                             [[P2re[:, j2, s, :] for s in range(2)] for j2 in range(2)],
                             [[P2im[:, j2, s, :] for s in range(2)] for j2 in range(2)])

                def postn_img(Unre, Unim, dre, dim, im):
                    # Un paired [128, 2img, 4nc, 512], written per image
                    img_dft2(dre, dim,
                             [[Unre[:, im, 0, :], Unre[:, im, 1, :]],
                              [Unre[:, im, 2, :], Unre[:, im, 3, :]]],
                             [[Unim[:, im, 0, :], Unim[:, im, 1, :]],
                              [Unim[:, im, 2, :], Unim[:, im, 3, :]]])

                def postm(Unre, Unim):
                    Ure = uupool.tile([NB, 2, 4, RES], f16, tag="ur")
                    Uim = uupool.tile([NB, 2, 4, RES], f16, tag="ui")
                    lo = slice(0, 256)
                    hi = slice(256, 512)
                    nc.vector.tensor_tensor(Ure[:, :, :, lo], Unre[:, :, :, lo], Unre[:, :, :, hi], ADD)
                    nc.vector.tensor_tensor(Uim[:, :, :, lo], Unim[:, :, :, lo], Unim[:, :, :, hi], ADD)
                    nc.vector.tensor_tensor(Ure[:, :, :, hi], Unre[:, :, :, lo], Unre[:, :, :, hi], SUB)
                    nc.vector.tensor_tensor(Uim[:, :, :, hi], Unim[:, :, :, lo], Unim[:, :, :, hi], SUB)
                    return Ure, Uim

                def mask_mults(Ure, Uim, l):
                    t1 = scp.tile([NB, 2, 4, RES], f16, tag="t1")
                    t2 = scp.tile([NB, 2, 4, RES], f16, tag="t2")
                    c = bc(pcos[l][:])
                    s = bc(psin[l][:])
                    nc.vector.tensor_tensor(t1[:], Ure[:], c, MULT)
                    nc.gpsimd.tensor_tensor(t2[:], Uim[:], s, MULT)
                    t3 = scp.tile([NB, 2, 4, RES], f16, tag="t1")
                    t4 = scp.tile([NB, 2, 4, RES], f16, tag="t2")
                    nc.vector.tensor_tensor(t3[:], Ure[:], s, MULT)
                    nc.vector.tensor_tensor(t4[:], Uim[:], c, MULT)
                    return t1, t2, t3, t4

                def mask_combine(t1, t2, t3, t4):
                    vre = vpool.tile([NB, 2, 4, RES], f16, tag="vr")
                    vim = vpool.tile([NB, 2, 4, RES], f16, tag="vi")
                    nc.vector.tensor_tensor(vre[:], t1[:], t2[:], SUB)
                    nc.vector.tensor_tensor(vim[:], t3[:], t4[:], ADD)
                    return vre, vim

                def entry(pr_i):
                    xt = xpool.tile([NB, 2, 4, RES], f16, tag="x")
                    nc.sync.dma_start(xt[:, 0, :, :], x_d[2 * pr_i])
                    nc.sync.dma_start(xt[:, 1, :, :], x_d[2 * pr_i + 1])
                    vre = vpool.tile([NB, 2, 4, RES], f16, tag="vr")
                    vim = vpool.tile([NB, 2, 4, RES], f16, tag="vi")
                    nc.vector.tensor_tensor(vre[:], xt[:], bc(pcos[0][:]), MULT)
                    nc.vector.tensor_tensor(vim[:], xt[:], bc(psin[0][:]), MULT)
                    return vre, vim

                def exit_intensity(Ure, Uim, pr_i):
                    s1 = scp.tile([NB, 2, 4, RES], f16, tag="t1")
                    s2 = scp.tile([NB, 2, 4, RES], f16, tag="t2")
                    nc.scalar.activation(s1[:], Ure[:], SQUARE)
                    nc.scalar.activation(s2[:], Uim[:], SQUARE)
                    ft = qpool.tile([NB, 2, 2, 2, RES], f16, tag="p2r")
                    nc.vector.tensor_tensor(ft[:], s1[:], s2[:], ADD)
                    nc.sync.dma_start(
                        featd[:, 2 * pr_i:2 * pr_i + 2, :].rearrange(
                            "p i f -> p (i f)"),
                        ft[:].rearrange("p i j s m -> p (i j s m)"))

                def fc_chunks(p):
                    """FC pass for 4 images [4p..4p+4): yields 16 chunk
                    emitters + a finalizer (bias add + out DMA)."""
                    M = 4
                    NF = 4 * RES
                    CH = 128
                    ps_fc = psum.tile([NB, RES], f32, tag="bk")

                    def one(c):
                        def emit():
                            wch = fcwp.tile([NB, NCLS, CH], f16, tag="wch")
                            nc.sync.dma_start(wch[:], fcw_d[:, :, bass.ts(c, CH)])
                            fch = fcrp.tile([NB, M, CH], f16, tag="fch")
                            nc.sync.dma_start(fch[:], featd[:, p * M:(p + 1) * M,
                                                            bass.ts(c, CH)])
                            for k in range(CH):
                                f = c * CH + k
                                nc.tensor.matmul(ps_fc[0:M, 0:NCLS],
                                                 fch[:, :, k], wch[:, :, k],
                                                 start=(f == 0), stop=(f == NF - 1))
                        return emit
                    for c in range(NF // CH):
                        yield one(c)

                    def fin():
                        ob = fcop.tile([M, NCLS], f32, tag=f"ob{p}")
                        nc.vector.tensor_tensor(ob[:], ps_fc[0:M, 0:NCLS],
                                                fcb_t[0:M, :], ADD)
                        nc.sync.dma_start(out_d[p * M:(p + 1) * M], ob[:])
                    yield fin

                def chunk_H1(st):
                    Pre, Pim = pre1(st["vre"], st["vim"])
                    P2 = []
                    for im in range(2):
                        P2re = qpool.tile([NB, 2, 2, RES], f16, tag=f"p2r{im}")
                        P2im = qpool.tile([NB, 2, 2, RES], f16, tag=f"p2i{im}")
                        dre = scp.tile([NB, 4, RES], f16, tag=f"d_r{im}")
                        dim = scp.tile([NB, 4, RES], f16, tag=f"d_i{im}")
                        bk = mm_stage(lambda j, s, a, im=im:
                                      (Pre[:, im, j, s, bass.ts(a, NB)],
                                       Pim[:, im, j, s, bass.ts(a, NB)]))
                        drain(bk, dre, dim)
                        pre2_img(P2re, P2im, dre, dim)
                        P2.append((P2re, P2im))
                    st["P2"] = P2

                def chunk_H2mm(st, l):
                    P2 = st["P2"]
                    Unre = upool.tile([NB, 2, 4, RES], f16, tag="unr")
                    Unim = upool.tile([NB, 2, 4, RES], f16, tag="uni")
                    for im in range(2):
                        P2re, P2im = P2[im]
                        dre = scp.tile([NB, 4, RES], f16, tag=f"d_r{im}")
                        dim = scp.tile([NB, 4, RES], f16, tag=f"d_i{im}")
                        bk = mm_stage(lambda j, s, a, P2re=P2re, P2im=P2im:
                                      (P2re[:, j, s, bass.ts(a, NB)],
                                       P2im[:, j, s, bass.ts(a, NB)]))
                        drain(bk, dre, dim)
                        postn_img(Unre, Unim, dre, dim, im)
                    Ure, Uim = postm(Unre, Unim)
                    if l < NL - 1:
                        st["T4"] = mask_mults(Ure, Uim, l + 1)
                    st["U"] = (Ure, Uim)

                def chunk_BD(st, l):
                    if l < NL - 1:
                        st["vre"], st["vim"] = mask_combine(*st["T4"])
                    else:
                        exit_intensity(*st["U"], st["pr"])

                # dual-stream skewed pipeline: even pairs on stream X,
                # odd pairs on stream Y; X runs one chunk ahead so each
                # pair's boundary glue overlaps the other's matmul chunks.
                NP = PER_CORE // 2

                def pair_chunks(pr):
                    st = {"pr": pr}

                    def c_entry():
                        st["vre"], st["vim"] = entry(pr)
                    yield c_entry
                    for l in range(NL):
                        yield lambda: chunk_H1(st)
                        yield (lambda ll: lambda: chunk_H2mm(st, ll))(l)
                        yield (lambda ll: lambda: chunk_BD(st, ll))(l)

                def stream(pairs):
                    for pr in pairs:
                        yield from pair_chunks(pr)

                cx = [f for f in stream(range(0, NP, 2))]
                cy = [f for f in stream(range(1, NP, 2))]
                cx[0]()  # entry(p0)
                late_const_dmas()
                cx[1]()  # H1(p0, l0)
                ix, iy = 2, 0
                # pairs (2k, 2k+1) = images [4k, 4k+4): fc pass k ready when
                # X has finished pair 2k (chunk 10(k+1)) and Y pair 2k+1.
                fcq = []
                fc_launched = 0
                while ix < len(cx) or iy < len(cy) or fcq:
                    if iy < len(cy):
                        cy[iy]()
                        iy += 1
                    if ix < len(cx):
                        cx[ix]()
                        ix += 1
                    while (fc_launched < NP // 2
                           and ix >= 10 * (fc_launched + 1) + 2
                           and iy >= 10 * (fc_launched + 1)):
                        fcq.extend(fc_chunks(fc_launched))
                        fc_launched += 1
                    if fcq:
                        fcq.pop(0)()
                        if len(fcq) > 6:
                            fcq.pop(0)()
                while fc_launched < NP // 2:
                    for f in fc_chunks(fc_launched):
                        f()
                    fc_launched += 1

    nc.compile()
    nc_cache["nc"] = nc
    return nc, None


def _const_arrays(phases, fc_w, fc_b, _cache={}):
    import hashlib
    key = (hashlib.sha1(phases.tobytes()).hexdigest()
           + hashlib.sha1(fc_w.tobytes()).hexdigest()
           + hashlib.sha1(fc_b.tobytes()).hexdigest())
    if _cache.get("key") == key:
        return _cache["val"]
    m2re, m2im, m2imn = _host_constants()
    ph = phases.reshape(NL, 4, NB, RES).transpose(0, 2, 1, 3)
    pcos = np.ascontiguousarray(np.cos(ph), np.float16)
    psin = np.ascontiguousarray(np.sin(ph), np.float16)
    fcw = np.ascontiguousarray(
        fc_w.reshape(NCLS, 4, NB, RES).transpose(2, 0, 1, 3).reshape(NB, NCLS, 4 * RES)
    ).astype(np.float16)
    fcb_rep = np.ascontiguousarray(
        np.broadcast_to(fc_b[None, :], (PER_CORE, NCLS))).astype(np.float32)
    val = {"m2re": m2re, "m2im": m2im, "m2imn": m2imn, "pcos": pcos,
           "psin": psin, "fcw": fcw, "fcb": fcb_rep}
    _cache["key"] = key
    _cache["val"] = val
    return val


def _prepare_in_maps(x, phases, fc_w, fc_b):
    consts = _const_arrays(phases, fc_w, fc_b)
    xs = x[:, 0].reshape(x.shape[0], 4, NB, RES).transpose(0, 2, 1, 3).astype(np.float16)
    in_maps = []
    for c in range(N_CORES):
        shard = np.ascontiguousarray(xs[c * PER_CORE:(c + 1) * PER_CORE])
        in_maps.append({"x": shard, **consts})
    return in_maps




def kernel(x, phases, fc_w, fc_b):
    x = np.asarray(x, dtype=np.float32)
    phases = np.asarray(phases, dtype=np.float32)
    fc_w = np.asarray(fc_w, dtype=np.float32)
    fc_b = np.asarray(fc_b, dtype=np.float32)
    batch = x.shape[0]

    in_maps = _prepare_in_maps(x, phases, fc_w, fc_b)
    runner = _cached_runner()
    out_by_core = runner(in_maps)
    out = np.concatenate(out_by_core, axis=0)
    return out.astype(np.float32)


def _cached_runner(_cache={}):
    """Build (once) a donated sharded jit wrapper around the Bass module."""
    if "fn" in _cache:
        return _cache["fn"]
    import jax
    import concourse.mybir as _mybir
    from concourse import bass2jax
    from jax.sharding import Mesh, PartitionSpec
    from jax.experimental.shard_map import shard_map

    nc, _ = _build()
    bass2jax.install_neuronx_cc_hook()
    pname = nc.partition_id_tensor.name if nc.partition_id_tensor else None
    in_names, out_names, out_avals = [], [], []
    for alloc in nc.m.functions[0].allocations:
        if not isinstance(alloc, _mybir.MemoryLocationSet):
            continue
        name = alloc.memorylocations[0].name
        if alloc.kind == "ExternalInput":
            if name != pname:
                in_names.append(name)
        elif alloc.kind == "ExternalOutput":
            out_names.append(name)
            out_avals.append(jax.core.ShapedArray(
                tuple(alloc.tensor_shape), _mybir.dt.np(alloc.dtype)))
    n_params = len(in_names)
    all_in = in_names + out_names + ([pname] if pname else [])

    def _body(*args):
        ops = list(args)
        if pname:
            ops.append(bass2jax.partition_id_tensor())
        return tuple(bass2jax._bass_exec_p.bind(
            *ops, out_avals=tuple(out_avals), in_names=tuple(all_in),
            out_names=tuple(out_names), lowering_input_output_aliases=(),
            sim_require_finite=True, sim_require_nnan=True, nc=nc))

    mesh = Mesh(np.asarray(jax.devices()[:N_CORES]), ("core",))
    n_outs = len(out_names)
    sharded = jax.jit(
        shard_map(_body, mesh=mesh,
                  in_specs=(PartitionSpec("core"),) * (n_params + n_outs),
                  out_specs=(PartitionSpec("core"),) * n_outs,
                  check_rep=False),
        donate_argnums=tuple(range(n_params, n_params + n_outs)),
        keep_unused=True,
    )

    def run(in_maps):
        concat_in = [
            np.concatenate([np.asarray(in_maps[c][nm]) for c in range(N_CORES)],
                           axis=0)
            for nm in in_names
        ]
        zeros = [np.zeros((N_CORES * av.shape[0], *av.shape[1:]), av.dtype)
                 for av in out_avals]
        outs = sharded(*concat_in, *zeros)
        oi = out_names.index("out")
        full = np.asarray(outs[oi]).reshape(N_CORES, *out_avals[oi].shape)
        return [full[c] for c in range(N_CORES)]

    _cache["fn"] = run
    return run


def time_device(inputs, reps=20):
    """Wall-clock the sharded PJRT executable with device-resident inputs.

    Returns the best per-call time in ns (includes dispatch overhead, so an
    upper bound on HW exec time).
    """
    import time as _time
    import jax
    import concourse.mybir as _mybir
    from concourse import bass2jax
    from jax.sharding import Mesh, PartitionSpec, NamedSharding
    from jax.experimental.shard_map import shard_map

    x = np.asarray(inputs["x"], dtype=np.float32)
    in_maps = _prepare_in_maps(
        x, np.asarray(inputs["phases"], np.float32),
        np.asarray(inputs["fc_w"], np.float32),
        np.asarray(inputs["fc_b"], np.float32))

    nc, _ = _build()
    bass2jax.install_neuronx_cc_hook()
    partition_name = nc.partition_id_tensor.name if nc.partition_id_tensor else None

    in_names, out_names, out_avals = [], [], []
    for alloc in nc.m.functions[0].allocations:
        if not isinstance(alloc, _mybir.MemoryLocationSet):
            continue
        name = alloc.memorylocations[0].name
        if alloc.kind == "ExternalInput":
            if name != partition_name:
                in_names.append(name)
        elif alloc.kind == "ExternalOutput":
            out_names.append(name)
            out_avals.append(jax.core.ShapedArray(
                tuple(alloc.tensor_shape), _mybir.dt.np(alloc.dtype)))
    n_params = len(in_names)
    all_in_names = in_names + out_names
    if partition_name is not None:
        all_in_names = all_in_names + [partition_name]

    def _body(*args):
        operands = list(args)
        if partition_name is not None:
            operands.append(bass2jax.partition_id_tensor())
        outs = bass2jax._bass_exec_p.bind(
            *operands,
            out_avals=tuple(out_avals),
            in_names=tuple(all_in_names),
            out_names=tuple(out_names),
            lowering_input_output_aliases=(),
            sim_require_finite=True,
            sim_require_nnan=True,
            nc=nc,
        )
        return tuple(outs)

    devices = jax.devices()[:N_CORES]
    mesh = Mesh(np.asarray(devices), ("core",))
    n_outs = len(out_names)
    in_specs = (PartitionSpec("core"),) * (n_params + n_outs)
    out_specs = (PartitionSpec("core"),) * n_outs
    jit_kwargs = {}
    if not os.environ.get("DONN_NO_DONATE"):
        jit_kwargs["donate_argnums"] = tuple(
            range(n_params, n_params + n_outs))
    sharded = jax.jit(
        shard_map(_body, mesh=mesh, in_specs=in_specs, out_specs=out_specs,
                  check_rep=False),
        keep_unused=True,
        **jit_kwargs,
    )
    sh = NamedSharding(mesh, PartitionSpec("core"))
    concat_in = [
        jax.device_put(
            np.concatenate([np.asarray(in_maps[c][nm]) for c in range(N_CORES)], axis=0),
            sh)
        for nm in in_names
    ]
    zero_np = [np.zeros((N_CORES * av.shape[0], *av.shape[1:]), av.dtype)
               for av in out_avals]

    def one_call():
        return sharded(*concat_in, *[jax.device_put(z, sh) for z in zero_np])

    # warmup + sanity: output must be nonzero
    w = one_call()
    jax.block_until_ready(w)
    if not os.environ.get("DONN_NOFC"):
        assert float(np.abs(np.asarray(w[0])).max()) > 0.0, "kernel produced zeros"

    def run_async(k):
        t0 = _time.perf_counter()
        outs = [one_call() for _ in range(k)]
        jax.block_until_ready(outs)
        return _time.perf_counter() - t0

    # min-of-n at several batch sizes, then least-squares slope: robust to
    # the axon tunnel's large positive latency outliers.
    ks = [4, 54, 104]
    mins = []
    for k in ks:
        mins.append(min(run_async(k) for _ in range(6)))
    ks_a = np.asarray(ks, dtype=np.float64)
    ms_a = np.asarray(mins, dtype=np.float64)
    slope = float(np.polyfit(ks_a, ms_a, 1)[0])
    return slope * 1e9

